# revision 20
# baseline (speedup 1.0000x reference)
"""Causal self-attention on 8 trn2 NeuronCores.

Sharding (batch+head hint): core c handles batch b = c//2 (data parallel)
and head-group g = c%2 (8 of 16 heads; tensor-parallel slice of w_qkv
columns / w_out rows). Each core computes a full-batch-slice partial of the
output projection over its 512 head dims; the two partials per batch are
summed on gather (the "all-reduce after out_proj").

Kernel dataflow per core (S=2048 tokens, D=1024, 8 heads x 64):
  phase 1: x^T comes pre-transposed from the host. qT/kT = W^T @ x^T
           (stationary weights, moving tokens) in [64h, S] layouts; v is
           computed in NATURAL [tokens, vdim] orientation and copied into
           v1 blocks of 72 cols (64 v + ones col for the denominator
           trick + pad), with an extra 56-col gap after every 4th chunk so
           the 128-col padded stationary of a diagonal chunk never
           overlaps the next q-block's v chunks (avoids false deps when
           qkv(tb+1) is interleaved into attention(tb)).
  phase 2: per head PAIR (even head on PE rows 0:64, odd head on rows
           64:128 -> concurrent row-group execution), exact-causal flash
           attention in transposed orientation: scoresT[k,q] pair -> one
           2-bank PSUM tile; ONE ScalarE exp per pair; causal mask applied
           post-exp as a bf16 multiply on the diagonal chunks;
           out_hT[dh,q] += v1_blk.T @ probsT (ones row at partition 64
           accumulates the denominator); normalize via reciprocal (read
           straight from PSUM) + gpsimd partition_broadcast.
  phase 3: partial out = oT.T @ Wout (per 128-token chunk), PSUM->SBUF on
           VectorE, 4KB-row DMA stores.

Scheduling: ScalarE exp (~146us total) is slower per attention slot than
the PE's score+PV work, so attention alone starves the PE. The qkv GEMM
of block tb+1 and the deferred out-projections are interleaved INTO
attention(tb) as PE filler so the tensor engine never waits on Scalar.
"""
import numpy as np

B = 4
S = 2048
D = 1024
HG = 8           # heads per core
DH = 64
NCORES = 8
NB = S // 512    # 512-token q blocks
KC = D // 128    # contraction chunks over D
HW = 16 * 72 + 4 * 56   # v1 per-head stride: 16 chunks * 72 + 4 diag pads


def _voff(s):
    # v1 column offset of k-chunk s within a head's 1376-col region
    return 72 * s + 56 * (s // 4)


_CACHE = {}


def _build_nc():
    import concourse.bass as bass  # noqa
    import concourse.mybir as mybir
    import concourse.tile as tile
    from concourse import bacc

    F32 = mybir.dt.float32
    BF = mybir.dt.bfloat16
    Exp = mybir.ActivationFunctionType.Exp

    nc = bacc.Bacc("TRN2", target_bir_lowering=False, debug=False,
                   enable_asserts=False, num_devices=NCORES)
    xT_d = nc.dram_tensor("xT", [D, S], BF, kind="ExternalInput")
    wqkv_d = nc.dram_tensor("wqkv", [D, 3 * 512], BF, kind="ExternalInput")
    wout_d = nc.dram_tensor("wout", [512, D], BF, kind="ExternalInput")
    masks_d = nc.dram_tensor("masks", [128, 256], BF, kind="ExternalInput")
    out_d = nc.dram_tensor("out", [S, D], BF, kind="ExternalOutput")

    with tile.TileContext(nc) as tc:
        with tc.tile_pool(name="persist", bufs=1) as persist, \
             tc.tile_pool(name="xT", bufs=2) as xT_pool, \
             tc.tile_pool(name="probs", bufs=7) as pr_pool, \
             tc.tile_pool(name="recip", bufs=2) as rc_pool, \
             tc.tile_pool(name="rbc", bufs=2) as rb_pool, \
             tc.tile_pool(name="obig", bufs=1) as obig_pool, \
             tc.tile_pool(name="ostage", bufs=3) as ost_pool, \
             tc.tile_pool(name="part", bufs=8) as part_pool, \
             tc.tile_pool(name="ps_sc", bufs=2, space="PSUM") as ps_sc, \
             tc.tile_pool(name="ps_wk", bufs=2, space="PSUM") as ps_wk, \
             tc.tile_pool(name="ps_out", bufs=2, space="PSUM") as ps_out:
            qT = persist.tile([128, 4 * S], BF)
            kT = persist.tile([128, 4 * S], BF)
            v1 = persist.tile([128, HG * HW], BF)
            oT = obig_pool.tile([128, 4 * S], BF)
            nc.vector.memset(v1[:], 1.0)
            tri2 = persist.tile([128, 256], BF)

            xT_tiles = {}

            def dma_x(tb):
                t = xT_pool.tile([128, KC * 512], BF, tag="xT",
                                 name=f"xTd_{tb}")
                for ki in range(KC):
                    nc.sync.dma_start(
                        t[:, ki * 512:(ki + 1) * 512],
                        xT_d[ki * 128:(ki + 1) * 128,
                             tb * 512:(tb + 1) * 512])
                xT_tiles[tb] = t

            # interleave x(0) and w-qk chunk DMAs so the first matmul chain
            # trickles in with the DMA stream; defer the w-v half + wout
            w_sb = persist.tile([128, KC * 1536], BF)
            xt0 = xT_pool.tile([128, KC * 512], BF, tag="xT", name="xTd_0")
            xT_tiles[0] = xt0
            for ki in range(KC):
                nc.sync.dma_start(
                    xt0[:, ki * 512:(ki + 1) * 512],
                    xT_d[ki * 128:(ki + 1) * 128, 0:512])
                if ki == 0:
                    # split so the very first matmul's weight dep is tiny
                    nc.sync.dma_start(
                        w_sb[:, 0:128], wqkv_d[0:128, 0:128])
                    nc.sync.dma_start(
                        w_sb[:, 128:1024], wqkv_d[0:128, 128:1024])
                else:
                    nc.sync.dma_start(
                        w_sb[:, ki * 1536: ki * 1536 + 1024],
                        wqkv_d[ki * 128:(ki + 1) * 128, 0:1024])
            wout_sb = persist.tile([128, 4 * D], BF)

            def dma_wv():
                for ki in range(KC):
                    nc.sync.dma_start(
                        w_sb[:, ki * 1536 + 1024:(ki + 1) * 1536],
                        wqkv_d[ki * 128:(ki + 1) * 128, 1024:1536])

            def dma_wout():
                nc.sync.dma_start(tri2[:], masks_d.ap())
                nc.sync.dma_start(
                    wout_sb[:].rearrange("p (k n) -> p k n", k=4),
                    wout_d.ap().rearrange("(k p) n -> p k n", p=128),
                )

            # --- qkv building blocks (emitted individually as fillers) ---
            def qk_group(tb, m):
                # q (m 0..3) / k (m 4..7): stationary w chunk, moving xT
                xT_sb = xT_tiles[tb]
                acc = ps_wk.tile([128, 512], F32, tag="wk",
                                 name=f"acc_{tb}_{m}")
                for ki in range(KC):
                    nc.tensor.matmul(
                        acc[:],
                        w_sb[:, ki * 1536 + m * 128: ki * 1536 + (m + 1) * 128],
                        xT_sb[:, ki * 512:(ki + 1) * 512],
                        start=(ki == 0), stop=(ki == KC - 1))
                dst = qT if m < 4 else kT
                r = m if m < 4 else m - 4
                nc.vector.tensor_copy(
                    dst[:, r * S + tb * 512: r * S + tb * 512 + 512],
                    acc[:])

            def v_group(tb, t):
                # v natural: stationary xT token chunk, moving Wv
                xT_sb = xT_tiles[tb]
                vacc = ps_wk.tile([128, 512], F32, tag="wk",
                                  name=f"vacc_{tb}_{t}")
                for ki in range(KC):
                    nc.tensor.matmul(
                        vacc[:],
                        xT_sb[:, ki * 512 + t * 128: ki * 512 + (t + 1) * 128],
                        w_sb[:, ki * 1536 + 1024: ki * 1536 + 1536],
                        start=(ki == 0), stop=(ki == KC - 1))
                sck = tb * 4 + t
                off = _voff(sck)
                v1v = v1[:].rearrange("p (h u) -> p h u", h=HG)
                nc.vector.tensor_copy(
                    v1v[:, :, off:off + 64],
                    vacc[:].rearrange("p (h o u) -> p h o u", h=HG, o=1))

            def qkv_fillers(tb):
                fs = [lambda tb=tb, m=m: qk_group(tb, m) for m in range(8)]
                fs += [lambda tb=tb, t=t: v_group(tb, t) for t in range(4)]
                return fs

            def attention_block(tb, fillers=(), late_fillers=()):
                ns = 4 * tb + 4   # k chunks for this q block
                nd = 4 * tb       # non-diagonal chunk count (even)
                tri2v = tri2[:].rearrange("p (h u) -> p h u", h=2)
                out_ps = {}
                pr_refs = {}

                def emit_sc(r, s):
                    lo = max(128 * s - 512 * tb, 0)
                    n = 512 - lo
                    scp = ps_sc.tile([128, 1024], F32, tag="sc",
                                     name=f"sc_{tb}_{r}_{s}")
                    for half in range(2):
                        po = 64 * half
                        nc.tensor.matmul(
                            scp[:, half * 512: half * 512 + n],
                            kT[po:po + 64, r * S + s * 128: r * S + s * 128 + 128],
                            qT[po:po + 64,
                               r * S + 512 * tb + lo: r * S + 512 * (tb + 1)],
                            start=True, stop=True)
                    pr = pr_pool.tile([128, 1024], BF, tag="probs",
                                      bufs=7, name=f"pr_{tb}_{r}_{s}")
                    nc.scalar.activation(
                        pr[:].rearrange("p (h u) -> p h u", h=2)[:, :, 0:n],
                        scp[:].rearrange("p (h u) -> p h u", h=2)[:, :, 0:n],
                        Exp)
                    if s >= nd:  # diagonal chunk: causal mask multiply
                        prv = pr[:].rearrange("p (h u) -> p h u", h=2)
                        nc.vector.tensor_mul(
                            prv[:, :, 0:128], prv[:, :, 0:128], tri2v)
                    pr_refs[(r, s)] = pr

                def emit_out(r, s):
                    lo = max(128 * s - 512 * tb, 0)
                    n = 512 - lo
                    prt = pr_refs.pop((r, s))
                    for half in range(2):
                        h = 2 * r + half
                        if s == 0:
                            out_ps[h] = ps_out.tile(
                                [128, 512], F32, tag="o", name=f"ops_{tb}_{h}")
                        off = h * HW + _voff(s)
                        nc.tensor.matmul(
                            out_ps[h][:, lo:512],
                            v1[:, off: off + 128],
                            prt[:, half * 512: half * 512 + n],
                            start=(s == 0), stop=(s == ns - 1))
                    if s == ns - 1:
                        for half in range(2):
                            h = 2 * r + half
                            po = 64 * half
                            op = out_ps.pop(h)
                            den = rc_pool.tile([1, 512], F32, tag="den")
                            nc.vector.tensor_copy(den[:], op[64:65, :])
                            rc = rc_pool.tile([1, 512], F32, tag="rc")
                            nc.vector.reciprocal_approx_fast(rc[:], den[:])
                            rb = rb_pool.tile([64, 512], F32, tag="rb")
                            nc.gpsimd.partition_broadcast(rb[:], rc[:])
                            nc.vector.tensor_mul(
                                oT[po:po + 64,
                                   r * S + 512 * tb: r * S + 512 * tb + 512],
                                op[0:64, :], rb[:])

                # Slots processed in PAIRS: the two scores pairs of slots
                # (2j, 2j+1) are emitted back-to-back so each 64-row
                # LDWEIGHTS hides under the opposite row-group's running
                # matmul, and the 4 following PV matmuls chain with a
                # single exposed weight load.
                LA = 4
                slots = [(r, s) for r in range(4) for s in range(ns)]
                fillers = list(fillers)
                npair = len(slots) // 2
                nf = len(fillers)
                fi = 0
                for j in range(npair):
                    while fi < nf and int((fi + 0.5) * npair / nf) <= j:
                        fillers[fi]()
                        fi += 1
                    emit_sc(*slots[2 * j])
                    emit_sc(*slots[2 * j + 1])
                    if 2 * j - LA >= 0:
                        emit_out(*slots[2 * j - LA])
                        emit_out(*slots[2 * j - LA + 1])
                lf = list(late_fillers)
                li = 0
                nper = (len(lf) + LA - 1) // LA if lf else 0
                for rs in slots[-LA:]:
                    emit_out(*rs)
                    for _ in range(nper):
                        if li < len(lf):
                            lf[li]()
                            li += 1
                while fi < nf:
                    fillers[fi]()
                    fi += 1
                while li < len(lf):
                    lf[li]()
                    li += 1

            ost_tiles = {}

            def proj_half(m, half):
                if half == 0:
                    ost_tiles[m] = ost_pool.tile([128, 1024], BF, tag="ost",
                                                 name=f"ost_{m}")
                ost = ost_tiles[m]
                pso = ps_wk.tile([128, 512], F32, tag="wk",
                                 name=f"pso_{m}_{half}")
                for k in range(4):
                    nc.tensor.matmul(
                        pso[:],
                        oT[:, k * S + m * 128: k * S + m * 128 + 128],
                        wout_sb[:, k * D + half * 512: k * D + half * 512 + 512],
                        start=(k == 0), stop=(k == 3))
                nc.vector.tensor_copy(
                    ost[:, half * 512:(half + 1) * 512], pso[:])
                if half == 1:
                    nc.sync.dma_start(
                        out_d[m * 128:(m + 1) * 128, :],
                        ost_tiles.pop(m)[:])

            def proj_halves(ms):
                return [lambda m=m, h=h: proj_half(m, h)
                        for m in ms for h in range(2)]

            # tail-latency k-split for the last q block's projections:
            # k=0..2 (head-pair rows ready after att(3) r=2) accumulate
            # early as fillers; only the k=3 matmul + add + store remain
            # after the final normalize.
            parts = {}

            def proj_partial(m, half):
                pp = ps_wk.tile([128, 512], F32, tag="wk",
                                name=f"pp_{m}_{half}")
                for k in range(3):
                    nc.tensor.matmul(
                        pp[:],
                        oT[:, k * S + m * 128: k * S + m * 128 + 128],
                        wout_sb[:, k * D + half * 512: k * D + half * 512 + 512],
                        start=(k == 0), stop=(k == 2))
                pt = part_pool.tile([128, 512], F32, tag="part",
                                    name=f"part_{m}_{half}")
                nc.vector.tensor_copy(pt[:], pp[:])
                parts[(m, half)] = pt

            def proj_final(m, half):
                if half == 0:
                    ost_tiles[m] = ost_pool.tile([128, 1024], BF, tag="ost",
                                                 name=f"ostf_{m}")
                ost = ost_tiles[m]
                pso = ps_wk.tile([128, 512], F32, tag="wk",
                                 name=f"psof_{m}_{half}")
                nc.tensor.matmul(
                    pso[:],
                    oT[:, 3 * S + m * 128: 3 * S + m * 128 + 128],
                    wout_sb[:, 3 * D + half * 512: 3 * D + half * 512 + 512],
                    start=True, stop=True)
                nc.vector.tensor_add(
                    ost[:, half * 512:(half + 1) * 512], pso[:],
                    parts.pop((m, half))[:])
                nc.sync.dma_start(
                    out_d[m * 128:(m + 1) * 128, half * 512:(half + 1) * 512],
                    ost[:, half * 512:(half + 1) * 512])

            # --- schedule ---
            # qkv(0) runs standalone; qkv(tb+1) is interleaved into att(tb)
            # as PE filler (ScalarE exp is slower per attention slot than
            # the PE's own work there), proj of ready blocks fills att(3).
            for gi, f in enumerate(qkv_fillers(0)):
                f()
                if gi == 4:
                    dma_wv()
                if gi == 6:
                    dma_x(1)
                if gi == 7:
                    dma_wout()

            fill_plan = {
                0: [lambda: dma_x(2)] + qkv_fillers(1),
                1: [lambda: dma_x(3)] + qkv_fillers(2),
                2: qkv_fillers(3),
                3: proj_halves([0, 1, 2, 3, 4, 5, 6, 7, 8, 9, 10, 11]),
            }
            late3 = [lambda m=m, h=h: proj_partial(m, h)
                     for m in (12, 13, 14, 15) for h in range(2)]
            for tb in range(NB):
                attention_block(tb, fill_plan[tb],
                                late3 if tb == 3 else ())
            for m in (12, 13, 14, 15):
                for h in range(2):
                    proj_final(m, h)
    nc.compile()
    return nc


def _make_masks():
    # tri2[p, j*128+c] = 1.0 if c >= p else 0 (keep-mask for the two
    # halves of a diagonal-chunk probs pair)
    p = np.arange(128)[:, None]
    c = np.arange(128)[None, :]
    tri = (c >= p).astype(np.float32)
    return np.concatenate([tri, tri], axis=1)


def _make_in_maps(x, w_qkv, w_out):
    import ml_dtypes
    bf = ml_dtypes.bfloat16
    masks = _make_masks().astype(bf)
    scale = np.float32(DH ** -0.5)
    in_maps = []
    for c in range(NCORES):
        g = c % 2
        wq = w_qkv[:, g * 512:(g + 1) * 512] * scale
        wk = w_qkv[:, D + g * 512: D + (g + 1) * 512]
        wv = w_qkv[:, 2 * D + g * 512: 2 * D + (g + 1) * 512]
        in_maps.append({
            "xT": np.ascontiguousarray(x[c // 2].T).astype(bf),
            "wqkv": np.ascontiguousarray(np.concatenate([wq, wk, wv], axis=1)).astype(bf),
            "wout": np.ascontiguousarray(w_out[g * 512:(g + 1) * 512, :]).astype(bf),
            "masks": masks,
        })
    return in_maps


def kernel(x, w_qkv, w_out):
    from concourse.bass_utils import run_bass_kernel_spmd

    x = np.asarray(x, dtype=np.float32)
    w_qkv = np.asarray(w_qkv, dtype=np.float32)
    w_out = np.asarray(w_out, dtype=np.float32)
    assert x.shape == (B, S, D) and w_qkv.shape == (D, 3 * D) and w_out.shape == (D, D)

    if "nc" not in _CACHE:
        _CACHE["nc"] = _build_nc()
    nc = _CACHE["nc"]

    in_maps = _make_in_maps(x, w_qkv, w_out)
    res = run_bass_kernel_spmd(nc, in_maps, core_ids=list(range(NCORES)),
                               trace=False)
    out = np.empty((B, S, D), dtype=np.float32)
    for b in range(B):
        out[b] = (res.results[2 * b]["out"].astype(np.float32)
                  + res.results[2 * b + 1]["out"].astype(np.float32))
    return out


# revision 23
# speedup vs baseline: 1.0193x; 1.0193x over previous
"""Causal self-attention on 8 trn2 NeuronCores.

Sharding (batch+head hint): core c handles batch b = c//2 (data parallel)
and head-group g = c%2 (8 of 16 heads; tensor-parallel slice of w_qkv
columns / w_out rows). Each core computes a full-batch-slice partial of the
output projection over its 512 head dims; the two partials per batch are
summed on gather (the "all-reduce after out_proj").

Kernel dataflow per core (S=2048 tokens, D=1024, 8 heads x 64):
  phase 1: x^T comes pre-transposed from the host. qT/kT = W^T @ x^T
           (stationary weights, moving tokens) in [64h, S] layouts; v is
           computed in NATURAL [tokens, vdim] orientation and copied into
           v1 blocks of 72 cols (64 v + ones col for the denominator
           trick + pad), with an extra 56-col gap after every 4th chunk so
           the 128-col padded stationary of a diagonal chunk never
           overlaps the next q-block's v chunks (avoids false deps when
           qkv(tb+1) is interleaved into attention(tb)).
  phase 2: per head PAIR (even head on PE rows 0:64, odd head on rows
           64:128 -> concurrent row-group execution), exact-causal flash
           attention in transposed orientation: scoresT[k,q] pair -> one
           2-bank PSUM tile; ONE ScalarE exp per pair; causal mask applied
           post-exp as a bf16 multiply on the diagonal chunks;
           out_hT[dh,q] += v1_blk.T @ probsT (ones row at partition 64
           accumulates the denominator); normalize via reciprocal (read
           straight from PSUM) + gpsimd partition_broadcast.
  phase 3: partial out = oT.T @ Wout (per 128-token chunk), PSUM->SBUF on
           VectorE, 4KB-row DMA stores.

Scheduling: ScalarE exp (~146us total) is slower per attention slot than
the PE's score+PV work, so attention alone starves the PE. The qkv GEMM
of block tb+1 and the deferred out-projections are interleaved INTO
attention(tb) as PE filler so the tensor engine never waits on Scalar.
"""
import numpy as np

B = 4
S = 2048
D = 1024
HG = 8           # heads per core
DH = 64
NCORES = 8
NB = S // 512    # 512-token q blocks
KC = D // 128    # contraction chunks over D
HW = 16 * 72 + 4 * 56   # v1 per-head stride: 16 chunks * 72 + 4 diag pads


def _voff(s):
    # v1 column offset of k-chunk s within a head's 1376-col region
    return 72 * s + 56 * (s // 4)


_CACHE = {}


def _build_nc():
    import concourse.bass as bass  # noqa
    import concourse.mybir as mybir
    import concourse.tile as tile
    from concourse import bacc

    F32 = mybir.dt.float32
    BF = mybir.dt.bfloat16
    Exp = mybir.ActivationFunctionType.Exp

    nc = bacc.Bacc("TRN2", target_bir_lowering=False, debug=False,
                   enable_asserts=False, num_devices=NCORES)
    xT_d = nc.dram_tensor("xT", [D, S], BF, kind="ExternalInput")
    wqkv_d = nc.dram_tensor("wqkv", [D, 3 * 512], BF, kind="ExternalInput")
    wout_d = nc.dram_tensor("wout", [512, D], BF, kind="ExternalInput")
    masks_d = nc.dram_tensor("masks", [128, 256], BF, kind="ExternalInput")
    out_d = nc.dram_tensor("out", [S, D], BF, kind="ExternalOutput")

    with tile.TileContext(nc) as tc:
        with tc.tile_pool(name="persist", bufs=1) as persist, \
             tc.tile_pool(name="xT", bufs=2) as xT_pool, \
             tc.tile_pool(name="probs", bufs=7) as pr_pool, \
             tc.tile_pool(name="recip", bufs=2) as rc_pool, \
             tc.tile_pool(name="rbc", bufs=2) as rb_pool, \
             tc.tile_pool(name="obig", bufs=1) as obig_pool, \
             tc.tile_pool(name="ostage", bufs=3) as ost_pool, \
             tc.tile_pool(name="part", bufs=8) as part_pool, \
             tc.tile_pool(name="ps_sc", bufs=2, space="PSUM") as ps_sc, \
             tc.tile_pool(name="ps_wk", bufs=2, space="PSUM") as ps_wk, \
             tc.tile_pool(name="ps_out", bufs=2, space="PSUM") as ps_out:
            qT = persist.tile([128, 4 * S], BF)
            kT = persist.tile([128, 4 * S], BF)
            v1 = persist.tile([128, HG * HW], BF)
            oT = obig_pool.tile([128, 4 * S], BF)
            nc.vector.memset(v1[:], 1.0)
            tri2 = persist.tile([128, 256], BF)

            xT_tiles = {}

            def dma_x(tb):
                t = xT_pool.tile([128, KC * 512], BF, tag="xT",
                                 name=f"xTd_{tb}")
                for ki in range(KC):
                    nc.sync.dma_start(
                        t[:, ki * 512:(ki + 1) * 512],
                        xT_d[ki * 128:(ki + 1) * 128,
                             tb * 512:(tb + 1) * 512])
                xT_tiles[tb] = t

            # interleave x(0) and w-qk chunk DMAs so the first matmul chain
            # trickles in with the DMA stream; defer the w-v half + wout
            w_sb = persist.tile([128, KC * 1536], BF)
            xt0 = xT_pool.tile([128, KC * 512], BF, tag="xT", name="xTd_0")
            xT_tiles[0] = xt0
            for ki in range(KC):
                nc.sync.dma_start(
                    xt0[:, ki * 512:(ki + 1) * 512],
                    xT_d[ki * 128:(ki + 1) * 128, 0:512])
                if ki == 0:
                    # split so the very first matmul's weight dep is tiny
                    nc.sync.dma_start(
                        w_sb[:, 0:128], wqkv_d[0:128, 0:128])
                    nc.sync.dma_start(
                        w_sb[:, 128:1024], wqkv_d[0:128, 128:1024])
                else:
                    nc.sync.dma_start(
                        w_sb[:, ki * 1536: ki * 1536 + 1024],
                        wqkv_d[ki * 128:(ki + 1) * 128, 0:1024])
            wout_sb = persist.tile([128, 4 * D], BF)

            def dma_wv():
                for ki in range(KC):
                    nc.sync.dma_start(
                        w_sb[:, ki * 1536 + 1024:(ki + 1) * 1536],
                        wqkv_d[ki * 128:(ki + 1) * 128, 1024:1536])

            def dma_wout():
                nc.sync.dma_start(tri2[:], masks_d.ap())
                nc.sync.dma_start(
                    wout_sb[:].rearrange("p (k n) -> p k n", k=4),
                    wout_d.ap().rearrange("(k p) n -> p k n", p=128),
                )

            # --- qkv building blocks (emitted individually as fillers) ---
            def qk_group(tb, m):
                # q (m 0..3) / k (m 4..7): stationary w chunk, moving xT
                xT_sb = xT_tiles[tb]
                acc = ps_wk.tile([128, 512], F32, tag="wk",
                                 name=f"acc_{tb}_{m}")
                for ki in range(KC):
                    nc.tensor.matmul(
                        acc[:],
                        w_sb[:, ki * 1536 + m * 128: ki * 1536 + (m + 1) * 128],
                        xT_sb[:, ki * 512:(ki + 1) * 512],
                        start=(ki == 0), stop=(ki == KC - 1))
                dst = qT if m < 4 else kT
                r = m if m < 4 else m - 4
                nc.vector.tensor_copy(
                    dst[:, r * S + tb * 512: r * S + tb * 512 + 512],
                    acc[:])

            def v_group(tb, t):
                # v natural: stationary xT token chunk, moving Wv
                xT_sb = xT_tiles[tb]
                vacc = ps_wk.tile([128, 512], F32, tag="wk",
                                  name=f"vacc_{tb}_{t}")
                for ki in range(KC):
                    nc.tensor.matmul(
                        vacc[:],
                        xT_sb[:, ki * 512 + t * 128: ki * 512 + (t + 1) * 128],
                        w_sb[:, ki * 1536 + 1024: ki * 1536 + 1536],
                        start=(ki == 0), stop=(ki == KC - 1))
                sck = tb * 4 + t
                off = _voff(sck)
                v1v = v1[:].rearrange("p (h u) -> p h u", h=HG)
                nc.vector.tensor_copy(
                    v1v[:, :, off:off + 64],
                    vacc[:].rearrange("p (h o u) -> p h o u", h=HG, o=1))

            def qkv_fillers(tb):
                fs = [lambda tb=tb, m=m: qk_group(tb, m) for m in range(8)]
                fs += [lambda tb=tb, t=t: v_group(tb, t) for t in range(4)]
                return fs

            def attention_block(tb, fillers=(), late_fillers=()):
                ns = 4 * tb + 4   # k chunks for this q block
                nd = 4 * tb       # non-diagonal chunk count (even)
                tri2v = tri2[:].rearrange("p (h u) -> p h u", h=2)
                out_ps = {}
                pr_refs = {}

                def emit_sc(r, s):
                    lo = max(128 * s - 512 * tb, 0)
                    n = 512 - lo
                    scp = ps_sc.tile([128, 1024], F32, tag="sc",
                                     name=f"sc_{tb}_{r}_{s}")
                    for half in range(2):
                        po = 64 * half
                        nc.tensor.matmul(
                            scp[:, half * 512: half * 512 + n],
                            kT[po:po + 64, r * S + s * 128: r * S + s * 128 + 128],
                            qT[po:po + 64,
                               r * S + 512 * tb + lo: r * S + 512 * (tb + 1)],
                            start=True, stop=True)
                    pr = pr_pool.tile([128, 1024], BF, tag="probs",
                                      bufs=7, name=f"pr_{tb}_{r}_{s}")
                    nc.scalar.activation(
                        pr[:].rearrange("p (h u) -> p h u", h=2)[:, :, 0:n],
                        scp[:].rearrange("p (h u) -> p h u", h=2)[:, :, 0:n],
                        Exp)
                    if s >= nd:  # diagonal chunk: causal mask multiply
                        prv = pr[:].rearrange("p (h u) -> p h u", h=2)
                        nc.vector.tensor_mul(
                            prv[:, :, 0:128], prv[:, :, 0:128], tri2v)
                    pr_refs[(r, s)] = pr

                def emit_out(r, s):
                    lo = max(128 * s - 512 * tb, 0)
                    n = 512 - lo
                    prt = pr_refs.pop((r, s))
                    for half in range(2):
                        h = 2 * r + half
                        if s == 0:
                            out_ps[h] = ps_out.tile(
                                [128, 512], F32, tag="o", name=f"ops_{tb}_{h}")
                        off = h * HW + _voff(s)
                        nc.tensor.matmul(
                            out_ps[h][:, lo:512],
                            v1[:, off: off + 128],
                            prt[:, half * 512: half * 512 + n],
                            start=(s == 0), stop=(s == ns - 1))
                    if s == ns - 1:
                        for half in range(2):
                            h = 2 * r + half
                            po = 64 * half
                            op = out_ps.pop(h)
                            den = rc_pool.tile([1, 512], F32, tag="den")
                            nc.vector.tensor_copy(den[:], op[64:65, :])
                            rc = rc_pool.tile([1, 512], F32, tag="rc")
                            nc.vector.reciprocal_approx_fast(rc[:], den[:])
                            rb = rb_pool.tile([64, 512], F32, tag="rb")
                            nc.gpsimd.partition_broadcast(rb[:], rc[:])
                            nc.vector.tensor_mul(
                                oT[po:po + 64,
                                   r * S + 512 * tb: r * S + 512 * tb + 512],
                                op[0:64, :], rb[:])

                # Slots processed in PAIRS: the two scores pairs of slots
                # (2j, 2j+1) are emitted back-to-back so each 64-row
                # LDWEIGHTS hides under the opposite row-group's running
                # matmul, and the 4 following PV matmuls chain with a
                # single exposed weight load.
                LA = 4
                slots = [(r, s) for r in range(4) for s in range(ns)]
                fillers = list(fillers)
                npair = len(slots) // 2
                nf = len(fillers)
                fi = 0
                for j in range(npair):
                    while fi < nf and int((fi + 0.5) * npair / nf) <= j:
                        fillers[fi]()
                        fi += 1
                    emit_sc(*slots[2 * j])
                    emit_sc(*slots[2 * j + 1])
                    if 2 * j - LA >= 0:
                        emit_out(*slots[2 * j - LA])
                        emit_out(*slots[2 * j - LA + 1])
                # flush: late fillers BEFORE each remaining out so the PE
                # has queued work while the last slots' exp drains on
                # ScalarE (the PE queue is FIFO; work emitted after a
                # stalled PV would stall with it).
                lf = list(late_fillers)
                li = 0
                nper = (len(lf) + LA - 1) // LA if lf else 0
                for rs in slots[-LA:]:
                    for _ in range(nper):
                        if li < len(lf):
                            lf[li]()
                            li += 1
                    emit_out(*rs)
                while li < len(lf):
                    lf[li]()
                    li += 1
                while fi < nf:
                    fillers[fi]()
                    fi += 1

            ost_tiles = {}

            def proj_half(m, half):
                if half == 0:
                    ost_tiles[m] = ost_pool.tile([128, 1024], BF, tag="ost",
                                                 name=f"ost_{m}")
                ost = ost_tiles[m]
                pso = ps_wk.tile([128, 512], F32, tag="wk",
                                 name=f"pso_{m}_{half}")
                for k in range(4):
                    nc.tensor.matmul(
                        pso[:],
                        oT[:, k * S + m * 128: k * S + m * 128 + 128],
                        wout_sb[:, k * D + half * 512: k * D + half * 512 + 512],
                        start=(k == 0), stop=(k == 3))
                nc.vector.tensor_copy(
                    ost[:, half * 512:(half + 1) * 512], pso[:])
                if half == 1:
                    nc.sync.dma_start(
                        out_d[m * 128:(m + 1) * 128, :],
                        ost_tiles.pop(m)[:])

            def proj_halves(ms):
                return [lambda m=m, h=h: proj_half(m, h)
                        for m in ms for h in range(2)]

            # tail-latency k-split for the last q block's projections:
            # k=0..2 (head-pair rows ready after att(3) r=2) accumulate
            # early as fillers; only the k=3 matmul + add + store remain
            # after the final normalize.
            parts = {}

            def proj_partial(m, half):
                pp = ps_wk.tile([128, 512], F32, tag="wk",
                                name=f"pp_{m}_{half}")
                for k in range(3):
                    nc.tensor.matmul(
                        pp[:],
                        oT[:, k * S + m * 128: k * S + m * 128 + 128],
                        wout_sb[:, k * D + half * 512: k * D + half * 512 + 512],
                        start=(k == 0), stop=(k == 2))
                pt = part_pool.tile([128, 512], F32, tag="part",
                                    name=f"part_{m}_{half}")
                nc.vector.tensor_copy(pt[:], pp[:])
                parts[(m, half)] = pt

            def proj_final(m, half):
                if half == 0:
                    ost_tiles[m] = ost_pool.tile([128, 1024], BF, tag="ost",
                                                 name=f"ostf_{m}")
                ost = ost_tiles[m]
                pso = ps_wk.tile([128, 512], F32, tag="wk",
                                 name=f"psof_{m}_{half}")
                nc.tensor.matmul(
                    pso[:],
                    oT[:, 3 * S + m * 128: 3 * S + m * 128 + 128],
                    wout_sb[:, 3 * D + half * 512: 3 * D + half * 512 + 512],
                    start=True, stop=True)
                nc.vector.tensor_add(
                    ost[:, half * 512:(half + 1) * 512], pso[:],
                    parts.pop((m, half))[:])
                nc.sync.dma_start(
                    out_d[m * 128:(m + 1) * 128, half * 512:(half + 1) * 512],
                    ost[:, half * 512:(half + 1) * 512])

            # --- schedule ---
            # qkv(0) runs standalone; qkv(tb+1) is interleaved into att(tb)
            # as PE filler (ScalarE exp is slower per attention slot than
            # the PE's own work there), proj of ready blocks fills att(3).
            # tb=0 head: the DMA stream paces the PE (chunks arrive every
            # ~1.1us but one group consumes a chunk in 216ns). Process the
            # first 4 q-groups ki-major with 4 parallel accumulators so
            # each arriving (x, w) chunk pair feeds 4 matmuls.
            def qk_quad0():
                a0 = ps_wk.tile([128, 512], F32, tag="wk", name="q0a0")
                a1 = ps_wk.tile([128, 512], F32, tag="wk", name="q0a1")
                asc = ps_sc.tile([128, 1024], F32, tag="sc", name="q0asc")
                accs = [a0[:], a1[:], asc[:, 0:512], asc[:, 512:1024]]
                xT_sb = xT_tiles[0]
                for ki in range(KC):
                    for m in range(4):
                        nc.tensor.matmul(
                            accs[m],
                            w_sb[:, ki * 1536 + m * 128: ki * 1536 + (m + 1) * 128],
                            xT_sb[:, ki * 512:(ki + 1) * 512],
                            start=(ki == 0), stop=(ki == KC - 1))
                for m in range(4):
                    nc.vector.tensor_copy(
                        qT[:, m * S: m * S + 512], accs[m])

            qk_quad0()
            for gi, f in enumerate(qkv_fillers(0)[4:]):
                f()
                if gi == 0:
                    dma_wv()
                if gi == 1:
                    dma_x(1)
                if gi == 3:
                    dma_wout()

            fill_plan = {
                0: [lambda: dma_x(2)] + qkv_fillers(1),
                1: [lambda: dma_x(3)] + qkv_fillers(2),
                2: qkv_fillers(3),
                3: proj_halves([0, 1, 2, 3, 4, 5, 6, 7, 8, 9]),
            }
            late3 = (proj_halves([10, 11])
                     + [lambda m=m, h=h: proj_partial(m, h)
                        for m in (12, 13, 14, 15) for h in range(2)])
            for tb in range(NB):
                attention_block(tb, fill_plan[tb],
                                late3 if tb == 3 else ())
            for m in (12, 13, 14, 15):
                for h in range(2):
                    proj_final(m, h)
    nc.compile()
    return nc


def _make_masks():
    # tri2[p, j*128+c] = 1.0 if c >= p else 0 (keep-mask for the two
    # halves of a diagonal-chunk probs pair)
    p = np.arange(128)[:, None]
    c = np.arange(128)[None, :]
    tri = (c >= p).astype(np.float32)
    return np.concatenate([tri, tri], axis=1)


def _make_in_maps(x, w_qkv, w_out):
    import ml_dtypes
    bf = ml_dtypes.bfloat16
    masks = _make_masks().astype(bf)
    scale = np.float32(DH ** -0.5)
    in_maps = []
    for c in range(NCORES):
        g = c % 2
        wq = w_qkv[:, g * 512:(g + 1) * 512] * scale
        wk = w_qkv[:, D + g * 512: D + (g + 1) * 512]
        wv = w_qkv[:, 2 * D + g * 512: 2 * D + (g + 1) * 512]
        in_maps.append({
            "xT": np.ascontiguousarray(x[c // 2].T).astype(bf),
            "wqkv": np.ascontiguousarray(np.concatenate([wq, wk, wv], axis=1)).astype(bf),
            "wout": np.ascontiguousarray(w_out[g * 512:(g + 1) * 512, :]).astype(bf),
            "masks": masks,
        })
    return in_maps


def kernel(x, w_qkv, w_out):
    from concourse.bass_utils import run_bass_kernel_spmd

    x = np.asarray(x, dtype=np.float32)
    w_qkv = np.asarray(w_qkv, dtype=np.float32)
    w_out = np.asarray(w_out, dtype=np.float32)
    assert x.shape == (B, S, D) and w_qkv.shape == (D, 3 * D) and w_out.shape == (D, D)

    if "nc" not in _CACHE:
        _CACHE["nc"] = _build_nc()
    nc = _CACHE["nc"]

    in_maps = _make_in_maps(x, w_qkv, w_out)
    res = run_bass_kernel_spmd(nc, in_maps, core_ids=list(range(NCORES)),
                               trace=False)
    out = np.empty((B, S, D), dtype=np.float32)
    for b in range(B):
        out[b] = (res.results[2 * b]["out"].astype(np.float32)
                  + res.results[2 * b + 1]["out"].astype(np.float32))
    return out


# revision 28
# speedup vs baseline: 1.0287x; 1.0092x over previous
"""Causal self-attention on 8 trn2 NeuronCores.

Sharding (batch+head hint): core c handles batch b = c//2 (data parallel)
and head-group g = c%2 (8 of 16 heads; tensor-parallel slice of w_qkv
columns / w_out rows). Each core computes a full-batch-slice partial of the
output projection over its 512 head dims; the two partials per batch are
summed on gather (the "all-reduce after out_proj").

Kernel dataflow per core (S=2048 tokens, D=1024, 8 heads x 64):
  phase 1: x^T comes pre-transposed from the host. qT/kT = W^T @ x^T
           (stationary weights, moving tokens) in [64h, S] layouts; v is
           computed in NATURAL [tokens, vdim] orientation and copied into
           v1 blocks of 72 cols (64 v + ones col for the denominator
           trick + pad), with an extra 56-col gap after every 4th chunk so
           the 128-col padded stationary of a diagonal chunk never
           overlaps the next q-block's v chunks (avoids false deps when
           qkv(tb+1) is interleaved into attention(tb)).
  phase 2: per head PAIR (even head on PE rows 0:64, odd head on rows
           64:128 -> concurrent row-group execution), exact-causal flash
           attention in transposed orientation: scoresT[k,q] pair -> one
           2-bank PSUM tile; ONE ScalarE exp per pair; causal mask applied
           post-exp as a bf16 multiply on the diagonal chunks;
           out_hT[dh,q] += v1_blk.T @ probsT (ones row at partition 64
           accumulates the denominator); normalize via reciprocal (read
           straight from PSUM) + gpsimd partition_broadcast.
  phase 3: partial out = oT.T @ Wout (per 128-token chunk), PSUM->SBUF on
           VectorE, 4KB-row DMA stores.

Scheduling: ScalarE exp (~146us total) is slower per attention slot than
the PE's score+PV work, so attention alone starves the PE. The qkv GEMM
of block tb+1 and the deferred out-projections are interleaved INTO
attention(tb) as PE filler so the tensor engine never waits on Scalar.
"""
import numpy as np

B = 4
S = 2048
D = 1024
HG = 8           # heads per core
DH = 64
NCORES = 8
NB = S // 512    # 512-token q blocks
KC = D // 128    # contraction chunks over D
HW = 16 * 72 + 4 * 56   # v1 per-head stride: 16 chunks * 72 + 4 diag pads


def _voff(s):
    # v1 column offset of k-chunk s within a head's 1376-col region
    return 72 * s + 56 * (s // 4)


_CACHE = {}


def _build_nc():
    import concourse.bass as bass  # noqa
    import concourse.mybir as mybir
    import concourse.tile as tile
    from concourse import bacc

    F32 = mybir.dt.float32
    BF = mybir.dt.bfloat16
    Exp = mybir.ActivationFunctionType.Exp

    nc = bacc.Bacc("TRN2", target_bir_lowering=False, debug=False,
                   enable_asserts=False, num_devices=NCORES)
    xT_d = nc.dram_tensor("xT", [D, S], BF, kind="ExternalInput")
    wqkv_d = nc.dram_tensor("wqkv", [D, 3 * 512], BF, kind="ExternalInput")
    wout_d = nc.dram_tensor("wout", [512, D], BF, kind="ExternalInput")
    masks_d = nc.dram_tensor("masks", [128, 256], BF, kind="ExternalInput")
    out_d = nc.dram_tensor("out", [S, D], BF, kind="ExternalOutput")

    with tile.TileContext(nc) as tc:
        with tc.tile_pool(name="persist", bufs=1) as persist, \
             tc.tile_pool(name="xT", bufs=2) as xT_pool, \
             tc.tile_pool(name="probs", bufs=7) as pr_pool, \
             tc.tile_pool(name="recip", bufs=2) as rc_pool, \
             tc.tile_pool(name="rbc", bufs=2) as rb_pool, \
             tc.tile_pool(name="obig", bufs=1) as obig_pool, \
             tc.tile_pool(name="ostage", bufs=3) as ost_pool, \
             tc.tile_pool(name="part", bufs=8) as part_pool, \
             tc.tile_pool(name="ps_sc", bufs=2, space="PSUM") as ps_sc, \
             tc.tile_pool(name="ps_wk", bufs=2, space="PSUM") as ps_wk, \
             tc.tile_pool(name="ps_out", bufs=2, space="PSUM") as ps_out:
            qT = persist.tile([128, 4 * S], BF)
            kT = persist.tile([128, 4 * S], BF)
            v1 = persist.tile([128, HG * HW], BF)
            oT = obig_pool.tile([128, 4 * S], BF)
            nc.vector.memset(v1[:], 1.0)
            tri2 = persist.tile([128, 256], BF)

            xT_tiles = {}

            def dma_x(tb):
                t = xT_pool.tile([128, KC * 512], BF, tag="xT",
                                 name=f"xTd_{tb}")
                for ki in range(KC):
                    nc.sync.dma_start(
                        t[:, ki * 512:(ki + 1) * 512],
                        xT_d[ki * 128:(ki + 1) * 128,
                             tb * 512:(tb + 1) * 512])
                xT_tiles[tb] = t

            # interleave x(0) and w-qk chunk DMAs so the first matmul chain
            # trickles in with the DMA stream; defer the w-v half + wout
            w_sb = persist.tile([128, KC * 1536], BF)
            xt0 = xT_pool.tile([128, KC * 512], BF, tag="xT", name="xTd_0")
            xT_tiles[0] = xt0
            for ki in range(KC):
                nc.sync.dma_start(
                    xt0[:, ki * 512:(ki + 1) * 512],
                    xT_d[ki * 128:(ki + 1) * 128, 0:512])
                if ki == 0:
                    # split so the very first matmul's weight dep is tiny
                    nc.sync.dma_start(
                        w_sb[:, 0:128], wqkv_d[0:128, 0:128])
                    nc.sync.dma_start(
                        w_sb[:, 128:1024], wqkv_d[0:128, 128:1024])
                else:
                    nc.sync.dma_start(
                        w_sb[:, ki * 1536: ki * 1536 + 1024],
                        wqkv_d[ki * 128:(ki + 1) * 128, 0:1024])
            wout_sb = persist.tile([128, 4 * D], BF)

            def dma_wv():
                for ki in range(KC):
                    nc.sync.dma_start(
                        w_sb[:, ki * 1536 + 1024:(ki + 1) * 1536],
                        wqkv_d[ki * 128:(ki + 1) * 128, 1024:1536])

            def dma_wout():
                nc.sync.dma_start(tri2[:], masks_d.ap())
                nc.sync.dma_start(
                    wout_sb[:].rearrange("p (k n) -> p k n", k=4),
                    wout_d.ap().rearrange("(k p) n -> p k n", p=128),
                )

            # --- qkv building blocks (emitted individually as fillers) ---
            def qk_group(tb, m):
                # q (m 0..3) / k (m 4..7): stationary w chunk, moving xT
                xT_sb = xT_tiles[tb]
                acc = ps_wk.tile([128, 512], F32, tag="wk",
                                 name=f"acc_{tb}_{m}")
                for ki in range(KC):
                    nc.tensor.matmul(
                        acc[:],
                        w_sb[:, ki * 1536 + m * 128: ki * 1536 + (m + 1) * 128],
                        xT_sb[:, ki * 512:(ki + 1) * 512],
                        start=(ki == 0), stop=(ki == KC - 1))
                dst = qT if m < 4 else kT
                r = m if m < 4 else m - 4
                nc.vector.tensor_copy(
                    dst[:, r * S + tb * 512: r * S + tb * 512 + 512],
                    acc[:])

            def v_group(tb, t):
                # v natural: stationary xT token chunk, moving Wv
                xT_sb = xT_tiles[tb]
                vacc = ps_wk.tile([128, 512], F32, tag="wk",
                                  name=f"vacc_{tb}_{t}")
                for ki in range(KC):
                    nc.tensor.matmul(
                        vacc[:],
                        xT_sb[:, ki * 512 + t * 128: ki * 512 + (t + 1) * 128],
                        w_sb[:, ki * 1536 + 1024: ki * 1536 + 1536],
                        start=(ki == 0), stop=(ki == KC - 1))
                sck = tb * 4 + t
                off = _voff(sck)
                v1v = v1[:].rearrange("p (h u) -> p h u", h=HG)
                nc.vector.tensor_copy(
                    v1v[:, :, off:off + 64],
                    vacc[:].rearrange("p (h o u) -> p h o u", h=HG, o=1))

            def qkv_fillers(tb):
                fs = [lambda tb=tb, m=m: qk_group(tb, m) for m in range(8)]
                fs += [lambda tb=tb, t=t: v_group(tb, t) for t in range(4)]
                return fs

            def attention_block(tb, fillers=(), late_fillers=()):
                ns = 4 * tb + 4   # k chunks for this q block
                nd = 4 * tb       # non-diagonal chunk count (even)
                tri2v = tri2[:].rearrange("p (h u) -> p h u", h=2)
                out_ps = {}
                pr_refs = {}

                def emit_sc(r, s):
                    lo = max(128 * s - 512 * tb, 0)
                    n = 512 - lo
                    scp = ps_sc.tile([128, 1024], F32, tag="sc",
                                     name=f"sc_{tb}_{r}_{s}")
                    for half in range(2):
                        po = 64 * half
                        nc.tensor.matmul(
                            scp[:, half * 512: half * 512 + n],
                            kT[po:po + 64, r * S + s * 128: r * S + s * 128 + 128],
                            qT[po:po + 64,
                               r * S + 512 * tb + lo: r * S + 512 * (tb + 1)],
                            start=True, stop=True)
                    pr = pr_pool.tile([128, 1024], BF, tag="probs",
                                      bufs=7, name=f"pr_{tb}_{r}_{s}")
                    nc.scalar.activation(
                        pr[:].rearrange("p (h u) -> p h u", h=2)[:, :, 0:n],
                        scp[:].rearrange("p (h u) -> p h u", h=2)[:, :, 0:n],
                        Exp)
                    if s >= nd:  # diagonal chunk: causal mask multiply
                        prv = pr[:].rearrange("p (h u) -> p h u", h=2)
                        nc.vector.tensor_mul(
                            prv[:, :, 0:128], prv[:, :, 0:128], tri2v)
                    pr_refs[(r, s)] = pr

                def emit_out(r, s):
                    lo = max(128 * s - 512 * tb, 0)
                    n = 512 - lo
                    prt = pr_refs.pop((r, s))
                    for half in range(2):
                        h = 2 * r + half
                        if s == 0:
                            out_ps[h] = ps_out.tile(
                                [128, 512], F32, tag="o", name=f"ops_{tb}_{h}")
                        off = h * HW + _voff(s)
                        nc.tensor.matmul(
                            out_ps[h][:, lo:512],
                            v1[:, off: off + 128],
                            prt[:, half * 512: half * 512 + n],
                            start=(s == 0), stop=(s == ns - 1))
                    if s == ns - 1:
                        for half in range(2):
                            h = 2 * r + half
                            po = 64 * half
                            op = out_ps.pop(h)
                            den = rc_pool.tile([1, 512], F32, tag="den")
                            nc.vector.tensor_copy(den[:], op[64:65, :])
                            rc = rc_pool.tile([1, 512], F32, tag="rc")
                            nc.vector.reciprocal_approx_fast(rc[:], den[:])
                            rb = rb_pool.tile([64, 512], F32, tag="rb")
                            nc.gpsimd.partition_broadcast(rb[:], rc[:])
                            nc.vector.tensor_mul(
                                oT[po:po + 64,
                                   r * S + 512 * tb: r * S + 512 * tb + 512],
                                op[0:64, :], rb[:])

                # Slots processed in PAIRS: the two scores pairs of slots
                # (2j, 2j+1) are emitted back-to-back so each 64-row
                # LDWEIGHTS hides under the opposite row-group's running
                # matmul, and the 4 following PV matmuls chain with a
                # single exposed weight load.
                LA = 4
                slots = [(r, s) for r in range(4) for s in range(ns)]
                fillers = list(fillers)
                npair = len(slots) // 2
                nf = len(fillers)
                fi = 0
                for j in range(npair):
                    while fi < nf and int((fi + 0.5) * npair / nf) <= j:
                        fillers[fi]()
                        fi += 1
                    emit_sc(*slots[2 * j])
                    emit_sc(*slots[2 * j + 1])
                    if 2 * j - LA >= 0:
                        emit_out(*slots[2 * j - LA])
                        emit_out(*slots[2 * j - LA + 1])
                # flush: late fillers BEFORE each remaining out so the PE
                # has queued work while the last slots' exp drains on
                # ScalarE (the PE queue is FIFO; work emitted after a
                # stalled PV would stall with it).
                lf = list(late_fillers)
                li = 0
                nper = (len(lf) + LA - 1) // LA if lf else 0
                for rs in slots[-LA:]:
                    for _ in range(nper):
                        if li < len(lf):
                            lf[li]()
                            li += 1
                    emit_out(*rs)
                while li < len(lf):
                    lf[li]()
                    li += 1
                while fi < nf:
                    fillers[fi]()
                    fi += 1

            ost_tiles = {}

            def proj_half(m, half, late=False):
                if half == 0:
                    ost_tiles[m] = ost_pool.tile([128, 1024], BF, tag="ost",
                                                 name=f"ost_{m}")
                ost = ost_tiles[m]
                pso = ps_wk.tile([128, 512], F32, tag="wk",
                                 name=f"pso_{m}_{half}")
                for k in range(4):
                    nc.tensor.matmul(
                        pso[:],
                        oT[:, k * S + m * 128: k * S + m * 128 + 128],
                        wout_sb[:, k * D + half * 512: k * D + half * 512 + 512],
                        start=(k == 0), stop=(k == 3))
                # late (post-exp) copies go to the then-idle ScalarE so the
                # DVE queue stays clear for the final normalize muls
                eng = nc.scalar if late else nc.vector
                if late:
                    eng.copy(ost[:, half * 512:(half + 1) * 512], pso[:])
                else:
                    eng.tensor_copy(ost[:, half * 512:(half + 1) * 512],
                                    pso[:])
                if half == 1:
                    nc.sync.dma_start(
                        out_d[m * 128:(m + 1) * 128, :],
                        ost_tiles.pop(m)[:])

            def proj_halves(ms):
                return [lambda m=m, h=h: proj_half(m, h)
                        for m in ms for h in range(2)]

            # tail-latency k-split for the last q block's projections:
            # k=0..2 (head-pair rows ready after att(3) r=2) accumulate
            # early as fillers; only the k=3 matmul + add + store remain
            # after the final normalize.
            parts = {}

            def proj_partial(m, half):
                pp = ps_wk.tile([128, 512], F32, tag="wk",
                                name=f"pp_{m}_{half}")
                for k in range(3):
                    nc.tensor.matmul(
                        pp[:],
                        oT[:, k * S + m * 128: k * S + m * 128 + 128],
                        wout_sb[:, k * D + half * 512: k * D + half * 512 + 512],
                        start=(k == 0), stop=(k == 2))
                pt = part_pool.tile([128, 512], F32, tag="part",
                                    name=f"part_{m}_{half}")
                nc.scalar.copy(pt[:], pp[:])
                parts[(m, half)] = pt

            def proj_final(m, half):
                if half == 0:
                    ost_tiles[m] = ost_pool.tile([128, 1024], BF, tag="ost",
                                                 name=f"ostf_{m}")
                ost = ost_tiles[m]
                pso = ps_wk.tile([128, 512], F32, tag="wk",
                                 name=f"psof_{m}_{half}")
                nc.tensor.matmul(
                    pso[:],
                    oT[:, 3 * S + m * 128: 3 * S + m * 128 + 128],
                    wout_sb[:, 3 * D + half * 512: 3 * D + half * 512 + 512],
                    start=True, stop=True)
                nc.vector.tensor_add(
                    ost[:, half * 512:(half + 1) * 512], pso[:],
                    parts.pop((m, half))[:])
                nc.sync.dma_start(
                    out_d[m * 128:(m + 1) * 128, half * 512:(half + 1) * 512],
                    ost[:, half * 512:(half + 1) * 512])

            # --- schedule ---
            # qkv(0) runs standalone; qkv(tb+1) is interleaved into att(tb)
            # as PE filler (ScalarE exp is slower per attention slot than
            # the PE's own work there), proj of ready blocks fills att(3).
            # tb=0 head: the DMA stream paces the PE (chunks arrive every
            # ~1.1us but one group consumes a chunk in 216ns). Process the
            # first 4 q-groups ki-major with 4 parallel accumulators so
            # each arriving (x, w) chunk pair feeds 4 matmuls.
            def qk_quad0():
                a0 = ps_wk.tile([128, 512], F32, tag="wk", name="q0a0")
                a1 = ps_wk.tile([128, 512], F32, tag="wk", name="q0a1")
                asc = ps_sc.tile([128, 1024], F32, tag="sc", name="q0asc")
                accs = [a0[:], a1[:], asc[:, 0:512], asc[:, 512:1024]]
                xT_sb = xT_tiles[0]
                for ki in range(KC):
                    for m in range(4):
                        nc.tensor.matmul(
                            accs[m],
                            w_sb[:, ki * 1536 + m * 128: ki * 1536 + (m + 1) * 128],
                            xT_sb[:, ki * 512:(ki + 1) * 512],
                            start=(ki == 0), stop=(ki == KC - 1))
                for m in range(4):
                    nc.vector.tensor_copy(
                        qT[:, m * S: m * S + 512], accs[m])

            qk_quad0()
            for gi, f in enumerate(qkv_fillers(0)[4:]):
                f()
                if gi == 0:
                    dma_wv()
                if gi == 1:
                    dma_x(1)
                if gi == 3:
                    dma_wout()

            fill_plan = {
                0: [lambda: dma_x(2)] + qkv_fillers(1),
                1: [lambda: dma_x(3)] + qkv_fillers(2),
                2: qkv_fillers(3),
                3: proj_halves([0, 1, 2, 3, 4, 5, 6, 7, 8, 9]),
            }
            late3 = ([lambda m=m, h=h: proj_half(m, h, late=True)
                      for m in (10, 11) for h in range(2)]
                     + [lambda m=m, h=h: proj_partial(m, h)
                        for m in (12, 13, 14, 15) for h in range(2)])
            for tb in range(NB):
                attention_block(tb, fill_plan[tb],
                                late3 if tb == 3 else ())
            for m in (12, 13, 14, 15):
                for h in range(2):
                    proj_final(m, h)
    nc.compile()
    return nc


def _make_masks():
    # tri2[p, j*128+c] = 1.0 if c >= p else 0 (keep-mask for the two
    # halves of a diagonal-chunk probs pair)
    p = np.arange(128)[:, None]
    c = np.arange(128)[None, :]
    tri = (c >= p).astype(np.float32)
    return np.concatenate([tri, tri], axis=1)


def _make_in_maps(x, w_qkv, w_out):
    import ml_dtypes
    bf = ml_dtypes.bfloat16
    masks = _make_masks().astype(bf)
    scale = np.float32(DH ** -0.5)
    in_maps = []
    for c in range(NCORES):
        g = c % 2
        wq = w_qkv[:, g * 512:(g + 1) * 512] * scale
        wk = w_qkv[:, D + g * 512: D + (g + 1) * 512]
        wv = w_qkv[:, 2 * D + g * 512: 2 * D + (g + 1) * 512]
        in_maps.append({
            "xT": np.ascontiguousarray(x[c // 2].T).astype(bf),
            "wqkv": np.ascontiguousarray(np.concatenate([wq, wk, wv], axis=1)).astype(bf),
            "wout": np.ascontiguousarray(w_out[g * 512:(g + 1) * 512, :]).astype(bf),
            "masks": masks,
        })
    return in_maps


def kernel(x, w_qkv, w_out):
    from concourse.bass_utils import run_bass_kernel_spmd

    x = np.asarray(x, dtype=np.float32)
    w_qkv = np.asarray(w_qkv, dtype=np.float32)
    w_out = np.asarray(w_out, dtype=np.float32)
    assert x.shape == (B, S, D) and w_qkv.shape == (D, 3 * D) and w_out.shape == (D, D)

    if "nc" not in _CACHE:
        _CACHE["nc"] = _build_nc()
    nc = _CACHE["nc"]

    in_maps = _make_in_maps(x, w_qkv, w_out)
    res = run_bass_kernel_spmd(nc, in_maps, core_ids=list(range(NCORES)),
                               trace=False)
    out = np.empty((B, S, D), dtype=np.float32)
    for b in range(B):
        out[b] = (res.results[2 * b]["out"].astype(np.float32)
                  + res.results[2 * b + 1]["out"].astype(np.float32))
    return out
